# revision 16
# baseline (speedup 1.0000x reference)
"""
AdaptiveGridSelfAttention fused Trainium2 kernel.

Per batch (8 batches over 8 cores, pure data parallel):
  x: [C=64, H=256, W=256] f32
  y = x + grid_sa(x);  out = y + ffn(y)

Host-preprocessed weights (biases folded via augmented ones-row):
  M_aug  [65,65] = [wq|bq]^T [wk|bk]   (S = x_aug^T M_aug x_aug)
  wvT_aug[65,64] = [wv|bv]^T
  w1T_aug[65,256] = [w1|b1]^T;  w2T [256,64] = w2^T;  b2row [1,64]

Attention orientation: S[l,k] (key pixel l on partitions). Softmax over the
partition axis; column sums come from a ones-stationary matmul that also
broadcasts them to all 64 partitions, so reciprocal+normalize are plain
full-width vector ops. No transposes.

x is recast to bf16 into a *window-major* layout (64 px per window contiguous)
so every matmul operand AP has a single free dimension (walrus requires 1D
free on the stationary operand).

Phase split: all attention (exp) then all FFN (gelu) to avoid ACT table-set
reloads between exp and gelu.
"""

import numpy as np
import ml_dtypes

C = 64
H = 256
W = 256
GS = 8
STRIPE_H = GS
N_STRIPES = H // GS     # 32
N_GROUPS = 4            # 8-window groups per stripe
GRP = 512               # pixels per group
HID = 256
SPX = 2048              # pixels per stripe

_CACHE = {}


def _build():
    import concourse.bass as bass
    import concourse.tile as tile
    from concourse import bacc, mybir

    f32 = mybir.dt.float32
    bf16 = mybir.dt.bfloat16

    nc = bacc.Bacc("TRN2", target_bir_lowering=False, debug=False,
                   num_devices=8)

    x_d = nc.dram_tensor("x", [C, H, W], f32, kind="ExternalInput").ap()
    M_d = nc.dram_tensor("m_aug", [C + 1, C + 1], bf16, kind="ExternalInput").ap()
    wv_d = nc.dram_tensor("wvt_aug", [C + 1, C], bf16, kind="ExternalInput").ap()
    w1_d = nc.dram_tensor("w1t_aug", [C + 1, HID], bf16, kind="ExternalInput").ap()
    w2_d = nc.dram_tensor("w2t", [HID, C], bf16, kind="ExternalInput").ap()
    b2_d = nc.dram_tensor("b2col", [C, 1], f32, kind="ExternalInput").ap()
    out_d = nc.dram_tensor("out", [C, H, W], f32, kind="ExternalOutput").ap()

    GELU = mybir.ActivationFunctionType.Gelu_apprx_tanh
    EXP = mybir.ActivationFunctionType.Exp

    with tile.TileContext(nc) as tc:
        with (
            tc.tile_pool(name="const", bufs=1) as constp,
            tc.tile_pool(name="ybuf", bufs=1) as ybufp,
            tc.tile_pool(name="xin", bufs=3) as xinp,
            tc.tile_pool(name="xwin", bufs=2) as xwinp,
            tc.tile_pool(name="small", bufs=3) as smallp,
            tc.tile_pool(name="hbuf", bufs=3) as hbufp,
            tc.tile_pool(name="obuf", bufs=3) as obufp,
            tc.tile_pool(name="ps_a", bufs=2, space=bass.MemorySpace.PSUM) as ps_a,
            tc.tile_pool(name="ps_b", bufs=2, space=bass.MemorySpace.PSUM) as ps_b,
            tc.tile_pool(name="ps_m", bufs=1, space=bass.MemorySpace.PSUM) as ps_m,
            tc.tile_pool(name="ps_c", bufs=2, space=bass.MemorySpace.PSUM) as ps_c,
        ):
            # ---- constants ----
            m_aug = constp.tile([C + 1, C + 1], bf16)
            nc.sync.dma_start(m_aug[:], M_d[:])
            wvt = constp.tile([C + 1, C], bf16)
            nc.sync.dma_start(wvt[:], wv_d[:])
            w1t = constp.tile([C + 1, HID], bf16)
            nc.sync.dma_start(w1t[:], w1_d[:])
            w2ta = constp.tile([128, C], bf16)
            nc.sync.dma_start(w2ta[:], w2_d[0:128, :])
            w2tb = constp.tile([128, C], bf16)
            nc.sync.dma_start(w2tb[:], w2_d[128:256, :])
            b2c = constp.tile([C, 1], f32)
            nc.sync.dma_start(b2c[:], b2_d[:])
            ones64 = constp.tile([C, C], bf16)
            nc.gpsimd.memset(ones64[:], 1.0)

            # persistent y (post-attention residual), raster layout, flat
            y = ybufp.tile([C + 1, H * W], bf16)   # [65, 65536]
            nc.gpsimd.memset(y[C:C + 1, 0:H * W // 2], 1.0)
            nc.gpsimd.memset(y[C:C + 1, H * W // 2:], 1.0)

            # =================== phase 1: attention ===================
            for s in range(N_STRIPES):
                xin = xinp.tile([C, STRIPE_H, W], f32)
                nc.sync.dma_start(xin[:], x_d[:, s * GS:(s + 1) * GS, :])
                # window-major bf16 x with ones row: col w*64 + r*8 + c
                xr = xwinp.tile([C, SPX], bf16, tag="xr")
                nc.gpsimd.tensor_copy(xr[:], xin[:].rearrange("p r c -> p (r c)"))
                xw = xwinp.tile([C + 1, SPX], bf16)
                nc.gpsimd.tensor_copy(
                    xw[0:C, :].rearrange("p (w r c) -> p w r c", w=32, r=8, c=8),
                    xin[:].rearrange("p r (w c) -> p w r c", w=32, c=8))
                if s < 2:
                    # xw pool has bufs=2; slots rotate so the ones row written
                    # in the first two iterations persists in both slots
                    nc.gpsimd.memset(xw[C:C + 1, :], 1.0)

                ybase = s * SPX
                for g in range(N_GROUPS):
                    c0 = g * GRP
                    xg = xw[:, c0:c0 + GRP]

                    # t2[j,k] = sum_i M_aug[i,j] x_aug[i,k]
                    t2p = ps_a.tile([C + 1, GRP], f32, tag="a")
                    nc.tensor.matmul(t2p[:], m_aug[:], xg, start=True, stop=True)
                    t2s = smallp.tile([C + 1, GRP], bf16, tag="t2s")
                    nc.scalar.copy(t2s[:], t2p[:])

                    sp = ps_b.tile([C, GRP], f32, tag="b")
                    vtp = ps_m.tile([C, GRP], f32, tag="vtp")
                    for w in range(8):
                        xww = xw[:, c0 + C * w:c0 + C * (w + 1)]
                        nc.tensor.matmul(sp[:, C * w:C * (w + 1)],
                                         xww, t2s[:, C * w:C * (w + 1)],
                                         start=True, stop=True)
                        nc.tensor.matmul(vtp[:, C * w:C * (w + 1)],
                                         xww, wvt[:], start=True, stop=True)

                    # P = exp(S/8); logits ~ N(0,1): no max subtraction needed
                    pexp = smallp.tile([C, GRP], bf16, tag="pexp")
                    nc.scalar.activation(pexp[:], sp[:], EXP, scale=0.125)

                    vts = smallp.tile([C, GRP], bf16, tag="vts")
                    nc.scalar.copy(vts[:], vtp[:])

                    # column sums broadcast to all 64 partitions via ones matmul
                    sbp = ps_m.tile([C, GRP], f32, tag="sbp")
                    nc.tensor.matmul(sbp[:], ones64[:], pexp[:], start=True, stop=True)
                    s_s = smallp.tile([C, GRP], f32, tag="ss")
                    nc.vector.tensor_copy(s_s[:], sbp[:])
                    rbc = smallp.tile([C, GRP], f32, tag="rbc")
                    nc.vector.reciprocal_approx_fast(rbc[:], s_s[:])

                    # out2[c,k] = sum_l vT[l,c] P[l,k]
                    o2p = ps_c.tile([C, GRP], f32, tag="c")
                    for w in range(8):
                        nc.tensor.matmul(o2p[:, C * w:C * (w + 1)],
                                         vts[:, C * w:C * (w + 1)],
                                         pexp[:, C * w:C * (w + 1)],
                                         start=True, stop=True)

                    # normalize, writing attn in group-raster order
                    attn = smallp.tile([C, GRP], bf16, tag="attn")
                    nc.vector.tensor_mul(
                        attn[:].rearrange("p (r w c) -> p w r c", r=8, w=8, c=8),
                        o2p[:], rbc[:])

                    # y = x + attn (all bf16, step-1 innermost)
                    y_stripe = y[0:C, ybase:ybase + SPX].rearrange(
                        "p (r q) -> p r q", r=8, q=W)
                    xr_st = xr[:].rearrange("p (r q) -> p r q", r=8, q=W)
                    nc.vector.tensor_add(
                        y_stripe[:, :, g * C:(g + 1) * C],
                        attn[:].rearrange("p (r k) -> p r k", r=8, k=C),
                        xr_st[:, :, g * C:(g + 1) * C])

            # =================== phase 2: FFN ===================
            for s in range(N_STRIPES):
                for hh in range(4):
                    base = s * SPX + hh * GRP
                    yc = y[:, base:base + GRP]
                    h1p = ps_a.tile([128, GRP], f32, tag="a")
                    nc.tensor.matmul(h1p[:], w1t[:, 0:128], yc, start=True, stop=True)
                    h1s = hbufp.tile([128, GRP], bf16, tag="h1s")
                    nc.scalar.activation(h1s[:], h1p[:], GELU)

                    h2p = ps_b.tile([128, GRP], f32, tag="b")
                    nc.tensor.matmul(h2p[:], w1t[:, 128:256], yc, start=True, stop=True)
                    h2s = hbufp.tile([128, GRP], bf16, tag="h2s")
                    nc.scalar.activation(h2s[:], h2p[:], GELU)

                    y2p = ps_c.tile([C, GRP], f32, tag="c")
                    nc.tensor.matmul(y2p[:], w2ta[:], h1s[:], start=True, stop=False)
                    nc.tensor.matmul(y2p[:], w2tb[:], h2s[:], start=False, stop=True)

                    oc = obufp.tile([C, GRP], f32)
                    nc.vector.scalar_tensor_tensor(
                        oc[:], y2p[:], b2c[:], y[0:C, base:base + GRP],
                        op0=mybir.AluOpType.add, op1=mybir.AluOpType.add)
                    r0 = s * GS + hh * 2
                    nc.sync.dma_start(out_d[:, r0:r0 + 2, :],
                                      oc[:].rearrange("p (r c) -> p r c", r=2, c=W))

    nc.compile()
    return nc


def _prep_weights(wq, bq, wk, bk, wv, bv, w1, b1, w2, b2):
    bf = ml_dtypes.bfloat16
    wq_aug = np.concatenate([wq.astype(np.float64),
                             bq.astype(np.float64)[:, None]], axis=1)  # [64,65]
    wk_aug = np.concatenate([wk.astype(np.float64),
                             bk.astype(np.float64)[:, None]], axis=1)
    m_aug = (wq_aug.T @ wk_aug).astype(np.float32).astype(bf)          # [65,65]
    wvt_aug = np.concatenate([wv.astype(np.float32).T,
                              bv.astype(np.float32)[None, :]], axis=0).astype(bf)
    w1t_aug = np.concatenate([w1.astype(np.float32).T,
                              b1.astype(np.float32)[None, :]], axis=0).astype(bf)
    w2t = np.ascontiguousarray(w2.astype(np.float32).T).astype(bf)     # [256,64]
    b2col = np.ascontiguousarray(b2.astype(np.float32)[:, None])       # [64,1]
    return m_aug, wvt_aug, w1t_aug, w2t, b2col


def kernel(x, wq, bq, wk, bk, wv, bv, w1, b1, w2, b2, _trace=False):
    from concourse.bass_utils import run_bass_kernel_spmd

    if "nc" not in _CACHE:
        _CACHE["nc"] = _build()
    nc = _CACHE["nc"]

    m_aug, wvt_aug, w1t_aug, w2t, b2col = _prep_weights(
        wq, bq, wk, bk, wv, bv, w1, b1, w2, b2)

    x = np.asarray(x, dtype=np.float32)
    B = x.shape[0]
    in_maps = []
    for i in range(8):
        in_maps.append({
            "x": np.ascontiguousarray(x[i % B]),
            "m_aug": m_aug, "wvt_aug": wvt_aug, "w1t_aug": w1t_aug,
            "w2t": w2t, "b2col": b2col,
        })

    res = run_bass_kernel_spmd(nc, in_maps, core_ids=list(range(8)),
                               trace=_trace)
    out = np.stack([np.asarray(res.results[i]["out"], dtype=np.float32)
                    for i in range(B)], axis=0)
    if _trace:
        return out, res
    return out


# revision 20
# speedup vs baseline: 1.3338x; 1.3338x over previous
"""
AdaptiveGridSelfAttention fused Trainium2 kernel.

Per batch (8 batches over 8 cores, pure data parallel):
  x: [C=64, H=256, W=256] f32
  y = x + grid_sa(x);  out = y + ffn(y)

Host-preprocessed weights (biases folded via augmented ones-row):
  M_aug  [65,65] = [wq|bq]^T [wk|bk]   (S = x_aug^T M_aug x_aug)
  wvT_aug[65,64] = [wv|bv]^T
  w1T_aug[65,256] = [w1|b1]^T;  w2T [256,64] = w2^T;  b2row [1,64]

Attention orientation: S[l,k] (key pixel l on partitions). Softmax over the
partition axis; column sums come from a ones-stationary matmul that also
broadcasts them to all 64 partitions, so reciprocal+normalize are plain
full-width vector ops. No transposes.

x is recast to bf16 into a *window-major* layout (64 px per window contiguous)
so every matmul operand AP has a single free dimension (walrus requires 1D
free on the stationary operand).

Phase split: all attention (exp) then all FFN (gelu) to avoid ACT table-set
reloads between exp and gelu.
"""

import numpy as np
import ml_dtypes

C = 64
H = 256
W = 256
GS = 8
STRIPE_H = GS
N_STRIPES = H // GS     # 32
N_GROUPS = 4            # 8-window groups per stripe
GRP = 512               # pixels per group
HID = 256
SPX = 2048              # pixels per stripe

_CACHE = {}


def _build():
    import concourse.bass as bass
    import concourse.tile as tile
    from concourse import bacc, mybir

    f32 = mybir.dt.float32
    bf16 = mybir.dt.bfloat16

    nc = bacc.Bacc("TRN2", target_bir_lowering=False, debug=False,
                   num_devices=8)

    x_d = nc.dram_tensor("x", [C, H, W], f32, kind="ExternalInput").ap()
    M_d = nc.dram_tensor("m_aug", [C + 1, C + 1], bf16, kind="ExternalInput").ap()
    wv_d = nc.dram_tensor("wvt_aug", [C + 1, C], bf16, kind="ExternalInput").ap()
    w1_d = nc.dram_tensor("w1t_aug", [C + 1, HID], bf16, kind="ExternalInput").ap()
    w2_d = nc.dram_tensor("w2t", [HID, C], bf16, kind="ExternalInput").ap()
    b2_d = nc.dram_tensor("b2col", [C, 1], f32, kind="ExternalInput").ap()
    out_d = nc.dram_tensor("out", [C, H, W], f32, kind="ExternalOutput").ap()

    GELU = mybir.ActivationFunctionType.Gelu_apprx_tanh
    EXP = mybir.ActivationFunctionType.Exp

    with tile.TileContext(nc) as tc:
        with (
            tc.tile_pool(name="const", bufs=1) as constp,
            tc.tile_pool(name="ybuf", bufs=1) as ybufp,
            tc.tile_pool(name="xin", bufs=3) as xinp,
            tc.tile_pool(name="xwin", bufs=2) as xwinp,
            tc.tile_pool(name="small", bufs=3) as smallp,
            tc.tile_pool(name="hbuf", bufs=3) as hbufp,
            tc.tile_pool(name="obuf", bufs=3) as obufp,
            tc.tile_pool(name="ps_a", bufs=2, space=bass.MemorySpace.PSUM) as ps_a,
            tc.tile_pool(name="ps_b", bufs=2, space=bass.MemorySpace.PSUM) as ps_b,
            tc.tile_pool(name="ps_m", bufs=1, space=bass.MemorySpace.PSUM) as ps_m,
            tc.tile_pool(name="ps_c", bufs=2, space=bass.MemorySpace.PSUM) as ps_c,
        ):
            # ---- constants ----
            m_aug = constp.tile([C + 1, C + 1], bf16)
            nc.sync.dma_start(m_aug[:], M_d[:])
            wvt = constp.tile([C + 1, C], bf16)
            nc.sync.dma_start(wvt[:], wv_d[:])
            w1t = constp.tile([C + 1, HID], bf16)
            nc.sync.dma_start(w1t[:], w1_d[:])
            w2ta = constp.tile([128, C], bf16)
            nc.sync.dma_start(w2ta[:], w2_d[0:128, :])
            w2tb = constp.tile([128, C], bf16)
            nc.sync.dma_start(w2tb[:], w2_d[128:256, :])
            b2c = constp.tile([C, 1], f32)
            nc.sync.dma_start(b2c[:], b2_d[:])
            ones128 = constp.tile([128, C], bf16)
            nc.gpsimd.memset(ones128[:], 1.0)

            # persistent y (post-attention residual), raster layout, flat
            y = ybufp.tile([C + 1, H * W], bf16)   # [65, 65536]
            nc.gpsimd.memset(y[C:C + 1, 0:H * W // 2], 1.0)
            nc.gpsimd.memset(y[C:C + 1, H * W // 2:], 1.0)

            # =================== phase 1: attention ===================
            for s in range(N_STRIPES):
                xin = xinp.tile([C, STRIPE_H, W], f32)
                nc.sync.dma_start(xin[:], x_d[:, s * GS:(s + 1) * GS, :])
                # window-major bf16 x with ones row: col w*64 + r*8 + c
                xw = xwinp.tile([C + 1, SPX], bf16)
                nc.vector.tensor_copy(
                    xw[0:C, :].rearrange("p (w r c) -> p w r c", w=32, r=8, c=8),
                    xin[:].rearrange("p r (w c) -> p w r c", w=32, c=8))
                if s < 2:
                    # xw pool has bufs=2; slots rotate so the ones row written
                    # in the first two iterations persists in both slots
                    nc.gpsimd.memset(xw[C:C + 1, :], 1.0)

                ybase = s * SPX
                for g in range(N_GROUPS):
                    c0 = g * GRP
                    xg = xw[:, c0:c0 + GRP]

                    # t2[j,k] = sum_i M_aug[i,j] x_aug[i,k]
                    t2p = ps_a.tile([C + 1, GRP], f32, tag="a")
                    nc.tensor.matmul(t2p[:], m_aug[:], xg, start=True, stop=True)
                    t2s = smallp.tile([C + 1, GRP], bf16, tag="t2s")
                    nc.scalar.copy(t2s[:], t2p[:])

                    # window pair p -> (w=p on partitions 0:64, w=p+4 on 64:128)
                    sp = ps_b.tile([128, GRP // 2], f32, tag="b")
                    vtp = ps_b.tile([128, GRP // 2], f32, tag="b")
                    for p in range(4):
                        for d in range(2):
                            w = p + 4 * d
                            xww = xw[:, c0 + C * w:c0 + C * (w + 1)]
                            tp = (0, 64) if d else (0, 0)
                            nc.tensor.matmul(sp[64 * d:64 * d + 64, C * p:C * (p + 1)],
                                             xww, t2s[:, C * w:C * (w + 1)],
                                             start=True, stop=True, tile_position=tp)
                            nc.tensor.matmul(vtp[64 * d:64 * d + 64, C * p:C * (p + 1)],
                                             xww, wvt[:], start=True, stop=True,
                                             tile_position=tp)

                    # P = exp(S/8); logits ~ N(0,1): no max subtraction needed
                    pexp = smallp.tile([128, GRP // 2], bf16, tag="pexp")
                    nc.scalar.activation(pexp[:], sp[:], EXP, scale=0.125)

                    vts = smallp.tile([128, GRP // 2], bf16, tag="vts")
                    nc.scalar.copy(vts[:], vtp[:])

                    # column sums: deck B re-decked to partitions 0:64 via
                    # tile_position=(64,0) (contraction rows 64-127, output cols 0-63)
                    sba = ps_m.tile([C, GRP // 2], f32, tag="sba")
                    sbb = ps_m.tile([C, GRP // 2], f32, tag="sbb")
                    nc.tensor.matmul(sba[:], ones128[0:64, :], pexp[0:64, :],
                                     start=True, stop=True, tile_position=(0, 0))
                    nc.tensor.matmul(sbb[:], ones128[64:128, :], pexp[64:128, :],
                                     start=True, stop=True, tile_position=(64, 0))
                    s_s = smallp.tile([C, GRP], f32, tag="ss")
                    nc.vector.tensor_copy(s_s[:, 0:256], sba[:])
                    nc.vector.tensor_copy(s_s[:, 256:512], sbb[:])
                    rbc = smallp.tile([C, GRP], f32, tag="rbc")
                    nc.vector.reciprocal_approx_fast(rbc[:], s_s[:])

                    # out2[c,k] = sum_l vT[l,c] P[l,k]; deck B re-decked likewise
                    o2a = ps_c.tile([C, GRP // 2], f32, tag="c")
                    o2b = ps_c.tile([C, GRP // 2], f32, tag="c")
                    for p in range(4):
                        nc.tensor.matmul(o2a[:, C * p:C * (p + 1)],
                                         vts[0:64, C * p:C * (p + 1)],
                                         pexp[0:64, C * p:C * (p + 1)],
                                         start=True, stop=True, tile_position=(0, 0))
                        nc.tensor.matmul(o2b[:, C * p:C * (p + 1)],
                                         vts[64:128, C * p:C * (p + 1)],
                                         pexp[64:128, C * p:C * (p + 1)],
                                         start=True, stop=True, tile_position=(64, 0))

                    # normalize each deck, writing attn in group-raster order
                    attn = smallp.tile([C, GRP], bf16, tag="attn")
                    attn_v = attn[:].rearrange("p (r w c) -> p w r c", r=8, w=8, c=8)
                    nc.vector.tensor_mul(attn_v[:, 0:4, :, :], o2a[:],
                                         rbc[:, 0:256])
                    nc.vector.tensor_mul(attn_v[:, 4:8, :, :], o2b[:],
                                         rbc[:, 256:512])

                    # y = x + attn (bf16, step-1 innermost)
                    y_stripe = y[0:C, ybase:ybase + SPX].rearrange(
                        "p (r q) -> p r q", r=8, q=W)
                    nc.vector.tensor_add(
                        y_stripe[:, :, g * C:(g + 1) * C]
                        .rearrange("p r (w c) -> p r w c", w=8, c=8),
                        attn[:].rearrange("p (r w c) -> p r w c", r=8, w=8, c=8),
                        xw[0:C, c0:c0 + GRP].rearrange(
                            "p (w r c) -> p r w c", w=8, r=8, c=8))

            # =================== phase 2: FFN ===================
            for s in range(N_STRIPES):
                for hh in range(4):
                    base = s * SPX + hh * GRP
                    yc = y[:, base:base + GRP]
                    h1p = ps_a.tile([128, GRP], f32, tag="a")
                    nc.tensor.matmul(h1p[:], w1t[:, 0:128], yc, start=True, stop=True)
                    h1s = hbufp.tile([128, GRP], bf16, tag="h1s")
                    nc.scalar.activation(h1s[:], h1p[:], GELU)

                    h2p = ps_b.tile([128, GRP], f32, tag="b")
                    nc.tensor.matmul(h2p[:], w1t[:, 128:256], yc, start=True, stop=True)
                    h2s = hbufp.tile([128, GRP], bf16, tag="h2s")
                    nc.scalar.activation(h2s[:], h2p[:], GELU)

                    y2p = ps_c.tile([C, GRP], f32, tag="c")
                    nc.tensor.matmul(y2p[:], w2ta[:], h1s[:], start=True, stop=False)
                    nc.tensor.matmul(y2p[:], w2tb[:], h2s[:], start=False, stop=True)

                    oc = obufp.tile([C, GRP], f32)
                    nc.vector.scalar_tensor_tensor(
                        oc[:], y2p[:], b2c[:], y[0:C, base:base + GRP],
                        op0=mybir.AluOpType.add, op1=mybir.AluOpType.add)
                    r0 = s * GS + hh * 2
                    nc.sync.dma_start(out_d[:, r0:r0 + 2, :],
                                      oc[:].rearrange("p (r c) -> p r c", r=2, c=W))

    nc.compile()
    return nc


def _prep_weights(wq, bq, wk, bk, wv, bv, w1, b1, w2, b2):
    bf = ml_dtypes.bfloat16
    wq_aug = np.concatenate([wq.astype(np.float64),
                             bq.astype(np.float64)[:, None]], axis=1)  # [64,65]
    wk_aug = np.concatenate([wk.astype(np.float64),
                             bk.astype(np.float64)[:, None]], axis=1)
    m_aug = (wq_aug.T @ wk_aug).astype(np.float32).astype(bf)          # [65,65]
    wvt_aug = np.concatenate([wv.astype(np.float32).T,
                              bv.astype(np.float32)[None, :]], axis=0).astype(bf)
    w1t_aug = np.concatenate([w1.astype(np.float32).T,
                              b1.astype(np.float32)[None, :]], axis=0).astype(bf)
    w2t = np.ascontiguousarray(w2.astype(np.float32).T).astype(bf)     # [256,64]
    b2col = np.ascontiguousarray(b2.astype(np.float32)[:, None])       # [64,1]
    return m_aug, wvt_aug, w1t_aug, w2t, b2col


def kernel(x, wq, bq, wk, bk, wv, bv, w1, b1, w2, b2, _trace=False):
    from concourse.bass_utils import run_bass_kernel_spmd

    if "nc" not in _CACHE:
        _CACHE["nc"] = _build()
    nc = _CACHE["nc"]

    m_aug, wvt_aug, w1t_aug, w2t, b2col = _prep_weights(
        wq, bq, wk, bk, wv, bv, w1, b1, w2, b2)

    x = np.asarray(x, dtype=np.float32)
    B = x.shape[0]
    in_maps = []
    for i in range(8):
        in_maps.append({
            "x": np.ascontiguousarray(x[i % B]),
            "m_aug": m_aug, "wvt_aug": wvt_aug, "w1t_aug": w1t_aug,
            "w2t": w2t, "b2col": b2col,
        })

    res = run_bass_kernel_spmd(nc, in_maps, core_ids=list(range(8)),
                               trace=_trace)
    out = np.stack([np.asarray(res.results[i]["out"], dtype=np.float32)
                    for i in range(B)], axis=0)
    if _trace:
        return out, res
    return out


# revision 22
# speedup vs baseline: 1.3508x; 1.0127x over previous
"""
AdaptiveGridSelfAttention fused Trainium2 kernel.

Per batch (8 batches over 8 cores, pure data parallel):
  x: [C=64, H=256, W=256] f32
  y = x + grid_sa(x);  out = y + ffn(y)

Host-preprocessed weights (biases folded via augmented ones-row):
  M_aug  [65,65] = [wq|bq]^T [wk|bk]   (S = x_aug^T M_aug x_aug)
  wvT_aug[65,64] = [wv|bv]^T
  w1T_aug[65,256] = [w1|b1]^T;  w2T [256,64] = w2^T;  b2row [1,64]

Attention orientation: S[l,k] (key pixel l on partitions). Softmax over the
partition axis; column sums come from a ones-stationary matmul that also
broadcasts them to all 64 partitions, so reciprocal+normalize are plain
full-width vector ops. No transposes.

x is recast to bf16 into a *window-major* layout (64 px per window contiguous)
so every matmul operand AP has a single free dimension (walrus requires 1D
free on the stationary operand).

Phase split: all attention (exp) then all FFN (gelu) to avoid ACT table-set
reloads between exp and gelu.
"""

import numpy as np
import ml_dtypes

C = 64
H = 256
W = 256
GS = 8
STRIPE_H = GS
N_STRIPES = H // GS     # 32
N_GROUPS = 4            # 8-window groups per stripe
GRP = 512               # pixels per group
HID = 256
SPX = 2048              # pixels per stripe

_CACHE = {}


def _build():
    import concourse.bass as bass
    import concourse.tile as tile
    from concourse import bacc, mybir

    f32 = mybir.dt.float32
    bf16 = mybir.dt.bfloat16

    nc = bacc.Bacc("TRN2", target_bir_lowering=False, debug=False,
                   num_devices=8)

    x_d = nc.dram_tensor("x", [C, H, W], f32, kind="ExternalInput").ap()
    M_d = nc.dram_tensor("m_aug", [C + 1, C + 1], bf16, kind="ExternalInput").ap()
    wv_d = nc.dram_tensor("wvt_aug", [C + 1, C], bf16, kind="ExternalInput").ap()
    w1_d = nc.dram_tensor("w1t_aug", [C + 1, HID], bf16, kind="ExternalInput").ap()
    w2_d = nc.dram_tensor("w2t", [HID, C], bf16, kind="ExternalInput").ap()
    b2_d = nc.dram_tensor("b2col", [C, 1], f32, kind="ExternalInput").ap()
    out_d = nc.dram_tensor("out", [C, H, W], f32, kind="ExternalOutput").ap()

    GELU = mybir.ActivationFunctionType.Gelu_apprx_tanh
    EXP = mybir.ActivationFunctionType.Exp

    with tile.TileContext(nc) as tc:
        with (
            tc.tile_pool(name="const", bufs=1) as constp,
            tc.tile_pool(name="ybuf", bufs=1) as ybufp,
            tc.tile_pool(name="xin", bufs=3) as xinp,
            tc.tile_pool(name="xwin", bufs=2) as xwinp,
            tc.tile_pool(name="small", bufs=3) as smallp,
            tc.tile_pool(name="hbuf", bufs=3) as hbufp,
            tc.tile_pool(name="obuf", bufs=3) as obufp,
            tc.tile_pool(name="ps_a", bufs=2, space=bass.MemorySpace.PSUM) as ps_a,
            tc.tile_pool(name="ps_b", bufs=2, space=bass.MemorySpace.PSUM) as ps_b,
            tc.tile_pool(name="ps_m", bufs=1, space=bass.MemorySpace.PSUM) as ps_m,
            tc.tile_pool(name="ps_c", bufs=2, space=bass.MemorySpace.PSUM) as ps_c,
        ):
            # ---- constants ----
            m_aug = constp.tile([C + 1, C + 1], bf16)
            nc.sync.dma_start(m_aug[:], M_d[:])
            wvt = constp.tile([C + 1, C], bf16)
            nc.sync.dma_start(wvt[:], wv_d[:])
            w1t = constp.tile([C + 1, HID], bf16)
            nc.sync.dma_start(w1t[:], w1_d[:])
            w2ta = constp.tile([128, C], bf16)
            nc.sync.dma_start(w2ta[:], w2_d[0:128, :])
            w2tb = constp.tile([128, C], bf16)
            nc.sync.dma_start(w2tb[:], w2_d[128:256, :])
            b2c = constp.tile([C, 1], f32)
            nc.sync.dma_start(b2c[:], b2_d[:])
            ones128 = constp.tile([128, C], bf16)
            nc.gpsimd.memset(ones128[:], 1.0)

            # persistent y (post-attention residual), raster layout, flat
            y = ybufp.tile([C + 1, H * W], bf16)   # [65, 65536]
            nc.gpsimd.memset(y[C:C + 1, 0:H * W // 2], 1.0)
            nc.gpsimd.memset(y[C:C + 1, H * W // 2:], 1.0)

            # =================== phase 1: attention ===================
            for s in range(N_STRIPES):
                xin = xinp.tile([C, STRIPE_H, W], f32)
                nc.sync.dma_start(xin[:], x_d[:, s * GS:(s + 1) * GS, :])
                # window-major bf16 x with ones row: col w*64 + r*8 + c
                xw = xwinp.tile([C + 1, SPX], bf16)
                xw_v4 = xw[0:C, :].rearrange("p (w r c) -> p w r c", w=32, r=8, c=8)
                xin_v4 = xin[:].rearrange("p r (w c) -> p w r c", w=32, c=8)
                nc.vector.tensor_copy(xw_v4[:, 0:16, :, :], xin_v4[:, 0:16, :, :])
                nc.scalar.copy(xw_v4[:, 16:32, :, :], xin_v4[:, 16:32, :, :])
                if s < 2:
                    # xw pool has bufs=2; slots rotate so the ones row written
                    # in the first two iterations persists in both slots
                    nc.gpsimd.memset(xw[C:C + 1, :], 1.0)

                ybase = s * SPX
                for g in range(N_GROUPS):
                    c0 = g * GRP
                    xg = xw[:, c0:c0 + GRP]

                    # t2[j,k] = sum_i M_aug[i,j] x_aug[i,k]
                    t2p = ps_a.tile([C + 1, GRP], f32, tag="a")
                    nc.tensor.matmul(t2p[:], m_aug[:], xg, start=True, stop=True)
                    t2s = smallp.tile([C + 1, GRP], bf16, tag="t2s")
                    nc.scalar.copy(t2s[:], t2p[:])

                    # window pair p -> (w=p on partitions 0:64, w=p+4 on 64:128)
                    sp = ps_b.tile([128, GRP // 2], f32, tag="b")
                    vtp = ps_b.tile([128, GRP // 2], f32, tag="b")
                    for p in range(4):
                        for d in range(2):
                            w = p + 4 * d
                            xww = xw[:, c0 + C * w:c0 + C * (w + 1)]
                            tp = (0, 64) if d else (0, 0)
                            nc.tensor.matmul(sp[64 * d:64 * d + 64, C * p:C * (p + 1)],
                                             xww, t2s[:, C * w:C * (w + 1)],
                                             start=True, stop=True, tile_position=tp)
                            nc.tensor.matmul(vtp[64 * d:64 * d + 64, C * p:C * (p + 1)],
                                             xww, wvt[:], start=True, stop=True,
                                             tile_position=tp)

                    # P = exp(S/8); logits ~ N(0,1): no max subtraction needed
                    pexp = smallp.tile([128, GRP // 2], bf16, tag="pexp")
                    nc.scalar.activation(pexp[:], sp[:], EXP, scale=0.125)

                    vts = smallp.tile([128, GRP // 2], bf16, tag="vts")
                    nc.scalar.copy(vts[:], vtp[:])

                    # column sums: deck B re-decked to partitions 0:64 via
                    # tile_position=(64,0) (contraction rows 64-127, output cols 0-63)
                    sba = ps_m.tile([C, GRP // 2], f32, tag="sba")
                    sbb = ps_m.tile([C, GRP // 2], f32, tag="sbb")
                    nc.tensor.matmul(sba[:], ones128[0:64, :], pexp[0:64, :],
                                     start=True, stop=True, tile_position=(0, 0))
                    nc.tensor.matmul(sbb[:], ones128[64:128, :], pexp[64:128, :],
                                     start=True, stop=True, tile_position=(64, 0))
                    s_s = smallp.tile([C, GRP], f32, tag="ss")
                    nc.vector.tensor_copy(s_s[:, 0:256], sba[:])
                    nc.vector.tensor_copy(s_s[:, 256:512], sbb[:])
                    rbc = smallp.tile([C, GRP], f32, tag="rbc")
                    nc.vector.reciprocal_approx_fast(rbc[:], s_s[:])

                    # out2[c,k] = sum_l vT[l,c] P[l,k]; deck B re-decked likewise
                    o2a = ps_c.tile([C, GRP // 2], f32, tag="c")
                    o2b = ps_c.tile([C, GRP // 2], f32, tag="c")
                    for p in range(4):
                        nc.tensor.matmul(o2a[:, C * p:C * (p + 1)],
                                         vts[0:64, C * p:C * (p + 1)],
                                         pexp[0:64, C * p:C * (p + 1)],
                                         start=True, stop=True, tile_position=(0, 0))
                        nc.tensor.matmul(o2b[:, C * p:C * (p + 1)],
                                         vts[64:128, C * p:C * (p + 1)],
                                         pexp[64:128, C * p:C * (p + 1)],
                                         start=True, stop=True, tile_position=(64, 0))

                    # normalize each deck, writing attn in group-raster order
                    attn = smallp.tile([C, GRP], bf16, tag="attn")
                    attn_v = attn[:].rearrange("p (r w c) -> p w r c", r=8, w=8, c=8)
                    nc.vector.tensor_mul(attn_v[:, 0:4, :, :], o2a[:],
                                         rbc[:, 0:256])
                    nc.vector.tensor_mul(attn_v[:, 4:8, :, :], o2b[:],
                                         rbc[:, 256:512])

                    # y = x + attn (bf16, step-1 innermost)
                    y_stripe = y[0:C, ybase:ybase + SPX].rearrange(
                        "p (r q) -> p r q", r=8, q=W)
                    nc.vector.tensor_add(
                        y_stripe[:, :, g * C:(g + 1) * C]
                        .rearrange("p r (w c) -> p r w c", w=8, c=8),
                        attn[:].rearrange("p (r w c) -> p r w c", r=8, w=8, c=8),
                        xw[0:C, c0:c0 + GRP].rearrange(
                            "p (w r c) -> p r w c", w=8, r=8, c=8))

            # =================== phase 2: FFN ===================
            for s in range(N_STRIPES):
                for hp in range(2):          # chunk pairs
                    bases = [s * SPX + (2 * hp + j) * GRP for j in range(2)]
                    ycs = [y[:, b:b + GRP] for b in bases]
                    h1p = [ps_a.tile([128, GRP], f32, tag="a", name=f"h1p{hp}_{j}") for j in range(2)]
                    h2p = [ps_b.tile([128, GRP], f32, tag="b", name=f"h2p{hp}_{j}") for j in range(2)]
                    h1s = [hbufp.tile([128, GRP], bf16, tag="h1s", name=f"h1s{hp}_{j}") for j in range(2)]
                    h2s = [hbufp.tile([128, GRP], bf16, tag="h2s", name=f"h2s{hp}_{j}") for j in range(2)]
                    for j in range(2):
                        nc.tensor.matmul(h1p[j][:], w1t[:, 0:128], ycs[j],
                                         start=True, stop=True)
                    for j in range(2):
                        nc.tensor.matmul(h2p[j][:], w1t[:, 128:256], ycs[j],
                                         start=True, stop=True)
                    for j in range(2):
                        nc.scalar.activation(h1s[j][:], h1p[j][:], GELU)
                        nc.scalar.activation(h2s[j][:], h2p[j][:], GELU)
                    y2p = [ps_c.tile([C, GRP], f32, tag="c", name=f"y2p{hp}_{j}") for j in range(2)]
                    for j in range(2):
                        nc.tensor.matmul(y2p[j][:], w2ta[:], h1s[j][:],
                                         start=True, stop=False)
                    for j in range(2):
                        nc.tensor.matmul(y2p[j][:], w2tb[:], h2s[j][:],
                                         start=False, stop=True)
                    for j in range(2):
                        oc = obufp.tile([C, GRP], f32)
                        nc.vector.scalar_tensor_tensor(
                            oc[:], y2p[j][:], b2c[:], y[0:C, bases[j]:bases[j] + GRP],
                            op0=mybir.AluOpType.add, op1=mybir.AluOpType.add)
                        r0 = s * GS + (2 * hp + j) * 2
                        nc.sync.dma_start(out_d[:, r0:r0 + 2, :],
                                          oc[:].rearrange("p (r c) -> p r c", r=2, c=W))

    nc.compile()
    return nc
def _prep_weights(wq, bq, wk, bk, wv, bv, w1, b1, w2, b2):
    bf = ml_dtypes.bfloat16
    wq_aug = np.concatenate([wq.astype(np.float64),
                             bq.astype(np.float64)[:, None]], axis=1)  # [64,65]
    wk_aug = np.concatenate([wk.astype(np.float64),
                             bk.astype(np.float64)[:, None]], axis=1)
    m_aug = (wq_aug.T @ wk_aug).astype(np.float32).astype(bf)          # [65,65]
    wvt_aug = np.concatenate([wv.astype(np.float32).T,
                              bv.astype(np.float32)[None, :]], axis=0).astype(bf)
    w1t_aug = np.concatenate([w1.astype(np.float32).T,
                              b1.astype(np.float32)[None, :]], axis=0).astype(bf)
    w2t = np.ascontiguousarray(w2.astype(np.float32).T).astype(bf)     # [256,64]
    b2col = np.ascontiguousarray(b2.astype(np.float32)[:, None])       # [64,1]
    return m_aug, wvt_aug, w1t_aug, w2t, b2col


def kernel(x, wq, bq, wk, bk, wv, bv, w1, b1, w2, b2, _trace=False):
    from concourse.bass_utils import run_bass_kernel_spmd

    if "nc" not in _CACHE:
        _CACHE["nc"] = _build()
    nc = _CACHE["nc"]

    m_aug, wvt_aug, w1t_aug, w2t, b2col = _prep_weights(
        wq, bq, wk, bk, wv, bv, w1, b1, w2, b2)

    x = np.asarray(x, dtype=np.float32)
    B = x.shape[0]
    in_maps = []
    for i in range(8):
        in_maps.append({
            "x": np.ascontiguousarray(x[i % B]),
            "m_aug": m_aug, "wvt_aug": wvt_aug, "w1t_aug": w1t_aug,
            "w2t": w2t, "b2col": b2col,
        })

    res = run_bass_kernel_spmd(nc, in_maps, core_ids=list(range(8)),
                               trace=_trace)
    out = np.stack([np.asarray(res.results[i]["out"], dtype=np.float32)
                    for i in range(B)], axis=0)
    if _trace:
        return out, res
    return out


# revision 25
# speedup vs baseline: 1.3515x; 1.0005x over previous
"""
AdaptiveGridSelfAttention fused Trainium2 kernel.

Per batch (8 batches over 8 cores, pure data parallel):
  x: [C=64, H=256, W=256] f32
  y = x + grid_sa(x);  out = y + ffn(y)

Host-preprocessed weights (biases folded via augmented ones-row):
  M_aug  [65,65] = [wq|bq]^T [wk|bk]   (S = x_aug^T M_aug x_aug)
  wvT_aug[65,64] = [wv|bv]^T
  w1T_aug[65,256] = [w1|b1]^T;  w2T [256,64] = w2^T;  b2row [1,64]

Attention orientation: S[l,k] (key pixel l on partitions). Softmax over the
partition axis; column sums come from a ones-stationary matmul that also
broadcasts them to all 64 partitions, so reciprocal+normalize are plain
full-width vector ops. No transposes.

x is recast to bf16 into a *window-major* layout (64 px per window contiguous)
so every matmul operand AP has a single free dimension (walrus requires 1D
free on the stationary operand).

Phase split: all attention (exp) then all FFN (gelu) to avoid ACT table-set
reloads between exp and gelu.
"""

import numpy as np
import ml_dtypes

C = 64
H = 256
W = 256
GS = 8
STRIPE_H = GS
N_STRIPES = H // GS     # 32
N_GROUPS = 4            # 8-window groups per stripe
GRP = 512               # pixels per group
HID = 256
SPX = 2048              # pixels per stripe

_CACHE = {}


def _patch_ldw_opt():
    # walrus is invoked with --enable-ldw-opt=false; enabling it lets codegen
    # use the background weight buffer so LDWEIGHTS overlaps matmuls
    import concourse.bass_utils as bu
    if getattr(bu, "_ldw_patched", False):
        return
    orig = bu.run_command

    def run_command_patched(argv, **kw):
        argv = [a
                for a in argv]
        return orig(argv, **kw)

    bu.run_command = run_command_patched
    bu._ldw_patched = True


def _build():
    import concourse.bass as bass
    import concourse.tile as tile
    from concourse import bacc, mybir

    _patch_ldw_opt()

    f32 = mybir.dt.float32
    bf16 = mybir.dt.bfloat16

    nc = bacc.Bacc("TRN2", target_bir_lowering=False, debug=False,
                   num_devices=8)

    x_d = nc.dram_tensor("x", [C, H, W], f32, kind="ExternalInput").ap()
    M_d = nc.dram_tensor("m_aug", [C + 1, C + 1], bf16, kind="ExternalInput").ap()
    wv_d = nc.dram_tensor("wvt_aug", [C + 1, C], bf16, kind="ExternalInput").ap()
    w1_d = nc.dram_tensor("w1t_aug", [C + 1, HID], bf16, kind="ExternalInput").ap()
    w2_d = nc.dram_tensor("w2t", [HID, C], bf16, kind="ExternalInput").ap()
    b2_d = nc.dram_tensor("b2col", [C, 1], f32, kind="ExternalInput").ap()
    out_d = nc.dram_tensor("out", [C, H, W], f32, kind="ExternalOutput").ap()

    GELU = mybir.ActivationFunctionType.Gelu_apprx_tanh
    EXP = mybir.ActivationFunctionType.Exp

    with tile.TileContext(nc) as tc:
        with (
            tc.tile_pool(name="const", bufs=1) as constp,
            tc.tile_pool(name="ybuf", bufs=1) as ybufp,
            tc.tile_pool(name="xin", bufs=3) as xinp,
            tc.tile_pool(name="xwin", bufs=2) as xwinp,
            tc.tile_pool(name="small", bufs=3) as smallp,
            tc.tile_pool(name="hbuf", bufs=3) as hbufp,
            tc.tile_pool(name="obuf", bufs=3) as obufp,
            tc.tile_pool(name="ps_a", bufs=2, space=bass.MemorySpace.PSUM) as ps_a,
            tc.tile_pool(name="ps_b", bufs=2, space=bass.MemorySpace.PSUM) as ps_b,
            tc.tile_pool(name="ps_m", bufs=1, space=bass.MemorySpace.PSUM) as ps_m,
            tc.tile_pool(name="ps_c", bufs=2, space=bass.MemorySpace.PSUM) as ps_c,
        ):
            # ---- constants ----
            m_aug = constp.tile([C + 1, C + 1], bf16)
            nc.sync.dma_start(m_aug[:], M_d[:])
            wvt = constp.tile([C + 1, C], bf16)
            nc.sync.dma_start(wvt[:], wv_d[:])
            w1t = constp.tile([C + 1, HID], bf16)
            nc.sync.dma_start(w1t[:], w1_d[:])
            w2ta = constp.tile([128, C], bf16)
            nc.sync.dma_start(w2ta[:], w2_d[0:128, :])
            w2tb = constp.tile([128, C], bf16)
            nc.sync.dma_start(w2tb[:], w2_d[128:256, :])
            b2c = constp.tile([C, 1], f32)
            nc.sync.dma_start(b2c[:], b2_d[:])
            ones128 = constp.tile([128, C], bf16)
            nc.gpsimd.memset(ones128[:], 1.0)

            # persistent y (post-attention residual), raster layout, flat
            y = ybufp.tile([C + 1, H * W], bf16)   # [65, 65536]
            nc.gpsimd.memset(y[C:C + 1, 0:H * W // 2], 1.0)
            nc.gpsimd.memset(y[C:C + 1, H * W // 2:], 1.0)

            # =================== phase 1: attention ===================
            for s in range(N_STRIPES):
                xin = xinp.tile([C, STRIPE_H, W], f32)
                nc.sync.dma_start(xin[:], x_d[:, s * GS:(s + 1) * GS, :])
                # window-major bf16 x with ones row: col w*64 + r*8 + c
                xw = xwinp.tile([C + 1, SPX], bf16)
                xw_v4 = xw[0:C, :].rearrange("p (w r c) -> p w r c", w=32, r=8, c=8)
                xin_v4 = xin[:].rearrange("p r (w c) -> p w r c", w=32, c=8)
                nc.vector.tensor_copy(xw_v4[:, 0:16, :, :], xin_v4[:, 0:16, :, :])
                nc.scalar.copy(xw_v4[:, 16:32, :, :], xin_v4[:, 16:32, :, :])
                if s < 2:
                    # xw pool has bufs=2; slots rotate so the ones row written
                    # in the first two iterations persists in both slots
                    nc.gpsimd.memset(xw[C:C + 1, :], 1.0)

                ybase = s * SPX
                for g in range(N_GROUPS):
                    c0 = g * GRP
                    xg = xw[:, c0:c0 + GRP]

                    # t2[j,k] = sum_i M_aug[i,j] x_aug[i,k]
                    t2p = ps_a.tile([C + 1, GRP], f32, tag="a")
                    nc.tensor.matmul(t2p[:], m_aug[:], xg, start=True, stop=True)
                    t2s = smallp.tile([C + 1, GRP], bf16, tag="t2s")
                    nc.scalar.copy(t2s[:], t2p[:])

                    # window pair p -> (w=p on partitions 0:64, w=p+4 on 64:128)
                    sp = ps_b.tile([128, GRP // 2], f32, tag="b")
                    vtp = ps_b.tile([128, GRP // 2], f32, tag="b")
                    for p in range(4):
                        for d in range(2):
                            w = p + 4 * d
                            xww = xw[:, c0 + C * w:c0 + C * (w + 1)]
                            tp = (0, 64) if d else (0, 0)
                            nc.tensor.matmul(sp[64 * d:64 * d + 64, C * p:C * (p + 1)],
                                             xww, t2s[:, C * w:C * (w + 1)],
                                             start=True, stop=True, tile_position=tp)
                            nc.tensor.matmul(vtp[64 * d:64 * d + 64, C * p:C * (p + 1)],
                                             xww, wvt[:], start=True, stop=True,
                                             tile_position=tp)

                    # P = exp(S/8); logits ~ N(0,1): no max subtraction needed
                    pexp = smallp.tile([128, GRP // 2], bf16, tag="pexp")
                    nc.scalar.activation(pexp[:], sp[:], EXP, scale=0.125)

                    vts = smallp.tile([128, GRP // 2], bf16, tag="vts")
                    nc.scalar.copy(vts[:], vtp[:])

                    # column sums: deck B re-decked to partitions 0:64 via
                    # tile_position=(64,0) (contraction rows 64-127, output cols 0-63)
                    sba = ps_m.tile([C, GRP // 2], f32, tag="sba")
                    sbb = ps_m.tile([C, GRP // 2], f32, tag="sbb")
                    nc.tensor.matmul(sba[:], ones128[0:64, :], pexp[0:64, :],
                                     start=True, stop=True, tile_position=(0, 0))
                    nc.tensor.matmul(sbb[:], ones128[64:128, :], pexp[64:128, :],
                                     start=True, stop=True, tile_position=(64, 0))
                    s_s = smallp.tile([C, GRP], f32, tag="ss")
                    nc.vector.tensor_copy(s_s[:, 0:256], sba[:])
                    nc.vector.tensor_copy(s_s[:, 256:512], sbb[:])
                    rbc = smallp.tile([C, GRP], f32, tag="rbc")
                    nc.vector.reciprocal_approx_fast(rbc[:], s_s[:])

                    # out2[c,k] = sum_l vT[l,c] P[l,k]; deck B re-decked likewise
                    o2a = ps_c.tile([C, GRP // 2], f32, tag="c")
                    o2b = ps_c.tile([C, GRP // 2], f32, tag="c")
                    for p in range(4):
                        nc.tensor.matmul(o2a[:, C * p:C * (p + 1)],
                                         vts[0:64, C * p:C * (p + 1)],
                                         pexp[0:64, C * p:C * (p + 1)],
                                         start=True, stop=True, tile_position=(0, 0))
                        nc.tensor.matmul(o2b[:, C * p:C * (p + 1)],
                                         vts[64:128, C * p:C * (p + 1)],
                                         pexp[64:128, C * p:C * (p + 1)],
                                         start=True, stop=True, tile_position=(64, 0))

                    # normalize each deck, writing attn in group-raster order
                    attn = smallp.tile([C, GRP], bf16, tag="attn")
                    attn_v = attn[:].rearrange("p (r w c) -> p w r c", r=8, w=8, c=8)
                    nc.vector.tensor_mul(attn_v[:, 0:4, :, :], o2a[:],
                                         rbc[:, 0:256])
                    nc.vector.tensor_mul(attn_v[:, 4:8, :, :], o2b[:],
                                         rbc[:, 256:512])

                    # y = x + attn (bf16, step-1 innermost)
                    y_stripe = y[0:C, ybase:ybase + SPX].rearrange(
                        "p (r q) -> p r q", r=8, q=W)
                    nc.vector.tensor_add(
                        y_stripe[:, :, g * C:(g + 1) * C]
                        .rearrange("p r (w c) -> p r w c", w=8, c=8),
                        attn[:].rearrange("p (r w c) -> p r w c", r=8, w=8, c=8),
                        xw[0:C, c0:c0 + GRP].rearrange(
                            "p (w r c) -> p r w c", w=8, r=8, c=8))

            # =================== phase 2: FFN ===================
            for s in range(N_STRIPES):
                for hp in range(2):          # chunk pairs
                    bases = [s * SPX + (2 * hp + j) * GRP for j in range(2)]
                    ycs = [y[:, b:b + GRP] for b in bases]
                    h1p = [ps_a.tile([128, GRP], f32, tag="a", name=f"h1pa{hp}"),
                           ps_m.tile([128, GRP], f32, tag="sba", name=f"h1pb{hp}")]
                    h2p = [ps_b.tile([128, GRP], f32, tag="b", name=f"h2pa{hp}"),
                           ps_m.tile([128, GRP], f32, tag="sbb", name=f"h2pb{hp}")]
                    h1s = [hbufp.tile([128, GRP], bf16, tag="h1s", name=f"h1s{hp}_{j}") for j in range(2)]
                    h2s = [hbufp.tile([128, GRP], bf16, tag="h2s", name=f"h2s{hp}_{j}") for j in range(2)]
                    for j in range(2):
                        nc.tensor.matmul(h1p[j][:], w1t[:, 0:128], ycs[j],
                                         start=True, stop=True)
                    for j in range(2):
                        nc.tensor.matmul(h2p[j][:], w1t[:, 128:256], ycs[j],
                                         start=True, stop=True)
                    for j in range(2):
                        nc.scalar.activation(h1s[j][:], h1p[j][:], GELU)
                        nc.scalar.activation(h2s[j][:], h2p[j][:], GELU)
                    y2p = [ps_c.tile([C, GRP], f32, tag="c", name=f"y2p{hp}_{j}") for j in range(2)]
                    for j in range(2):
                        nc.tensor.matmul(y2p[j][:], w2ta[:], h1s[j][:],
                                         start=True, stop=False)
                    for j in range(2):
                        nc.tensor.matmul(y2p[j][:], w2tb[:], h2s[j][:],
                                         start=False, stop=True)
                    for j in range(2):
                        oc = obufp.tile([C, GRP], f32)
                        nc.vector.scalar_tensor_tensor(
                            oc[:], y2p[j][:], b2c[:], y[0:C, bases[j]:bases[j] + GRP],
                            op0=mybir.AluOpType.add, op1=mybir.AluOpType.add)
                        r0 = s * GS + (2 * hp + j) * 2
                        nc.sync.dma_start(out_d[:, r0:r0 + 2, :],
                                          oc[:].rearrange("p (r c) -> p r c", r=2, c=W))

    nc.compile()
    return nc
def _prep_weights(wq, bq, wk, bk, wv, bv, w1, b1, w2, b2):
    bf = ml_dtypes.bfloat16
    wq_aug = np.concatenate([wq.astype(np.float64),
                             bq.astype(np.float64)[:, None]], axis=1)  # [64,65]
    wk_aug = np.concatenate([wk.astype(np.float64),
                             bk.astype(np.float64)[:, None]], axis=1)
    m_aug = (wq_aug.T @ wk_aug).astype(np.float32).astype(bf)          # [65,65]
    wvt_aug = np.concatenate([wv.astype(np.float32).T,
                              bv.astype(np.float32)[None, :]], axis=0).astype(bf)
    w1t_aug = np.concatenate([w1.astype(np.float32).T,
                              b1.astype(np.float32)[None, :]], axis=0).astype(bf)
    w2t = np.ascontiguousarray(w2.astype(np.float32).T).astype(bf)     # [256,64]
    b2col = np.ascontiguousarray(b2.astype(np.float32)[:, None])       # [64,1]
    return m_aug, wvt_aug, w1t_aug, w2t, b2col


def kernel(x, wq, bq, wk, bk, wv, bv, w1, b1, w2, b2, _trace=False):
    from concourse.bass_utils import run_bass_kernel_spmd

    if "nc" not in _CACHE:
        _CACHE["nc"] = _build()
    nc = _CACHE["nc"]

    m_aug, wvt_aug, w1t_aug, w2t, b2col = _prep_weights(
        wq, bq, wk, bk, wv, bv, w1, b1, w2, b2)

    x = np.asarray(x, dtype=np.float32)
    B = x.shape[0]
    in_maps = []
    for i in range(8):
        in_maps.append({
            "x": np.ascontiguousarray(x[i % B]),
            "m_aug": m_aug, "wvt_aug": wvt_aug, "w1t_aug": w1t_aug,
            "w2t": w2t, "b2col": b2col,
        })

    res = run_bass_kernel_spmd(nc, in_maps, core_ids=list(range(8)),
                               trace=_trace)
    out = np.stack([np.asarray(res.results[i]["out"], dtype=np.float32)
                    for i in range(B)], axis=0)
    if _trace:
        return out, res
    return out


# revision 26
# speedup vs baseline: 1.3850x; 1.0248x over previous
"""
AdaptiveGridSelfAttention fused Trainium2 kernel.

Per batch (8 batches over 8 cores, pure data parallel):
  x: [C=64, H=256, W=256] f32
  y = x + grid_sa(x);  out = y + ffn(y)

Host-preprocessed weights (biases folded via augmented ones-row):
  M_aug  [65,65] = [wq|bq]^T [wk|bk]   (S = x_aug^T M_aug x_aug)
  wvT_aug[65,64] = [wv|bv]^T
  w1T_aug[65,256] = [w1|b1]^T;  w2T [256,64] = w2^T;  b2row [1,64]

Attention orientation: S[l,k] (key pixel l on partitions). Softmax over the
partition axis; column sums come from a ones-stationary matmul that also
broadcasts them to all 64 partitions, so reciprocal+normalize are plain
full-width vector ops. No transposes.

x is recast to bf16 into a *window-major* layout (64 px per window contiguous)
so every matmul operand AP has a single free dimension (walrus requires 1D
free on the stationary operand).

Phase split: all attention (exp) then all FFN (gelu) to avoid ACT table-set
reloads between exp and gelu.
"""

import numpy as np
import ml_dtypes

C = 64
H = 256
W = 256
GS = 8
STRIPE_H = GS
N_STRIPES = H // GS     # 32
N_GROUPS = 4            # 8-window groups per stripe
GRP = 512               # pixels per group
HID = 256
SPX = 2048              # pixels per stripe

_CACHE = {}


def _patch_ldw_opt():
    # walrus is invoked with --enable-ldw-opt=false; enabling it lets codegen
    # use the background weight buffer so LDWEIGHTS overlaps matmuls
    import concourse.bass_utils as bu
    if getattr(bu, "_ldw_patched", False):
        return
    orig = bu.run_command

    def run_command_patched(argv, **kw):
        argv = [a
                for a in argv]
        return orig(argv, **kw)

    bu.run_command = run_command_patched
    bu._ldw_patched = True


def _build():
    import concourse.bass as bass
    import concourse.tile as tile
    from concourse import bacc, mybir

    _patch_ldw_opt()

    f32 = mybir.dt.float32
    bf16 = mybir.dt.bfloat16

    nc = bacc.Bacc("TRN2", target_bir_lowering=False, debug=False,
                   num_devices=8)

    x_d = nc.dram_tensor("x", [C, H, W], f32, kind="ExternalInput").ap()
    M_d = nc.dram_tensor("m_aug", [C + 1, C + 1], bf16, kind="ExternalInput").ap()
    wv_d = nc.dram_tensor("wvt_aug", [C + 1, C], bf16, kind="ExternalInput").ap()
    w1_d = nc.dram_tensor("w1t_aug", [C + 1, HID], bf16, kind="ExternalInput").ap()
    w2_d = nc.dram_tensor("w2t", [HID, C], bf16, kind="ExternalInput").ap()
    b2_d = nc.dram_tensor("b2col", [C, 1], f32, kind="ExternalInput").ap()
    out_d = nc.dram_tensor("out", [C, H, W], f32, kind="ExternalOutput").ap()

    GELU = mybir.ActivationFunctionType.Gelu_apprx_tanh
    EXP = mybir.ActivationFunctionType.Exp

    with tile.TileContext(nc) as tc:
        with (
            tc.tile_pool(name="const", bufs=1) as constp,
            tc.tile_pool(name="ybuf", bufs=1) as ybufp,
            tc.tile_pool(name="xin", bufs=3) as xinp,
            tc.tile_pool(name="xwin", bufs=2) as xwinp,
            tc.tile_pool(name="small", bufs=3) as smallp,
            tc.tile_pool(name="hbuf", bufs=3) as hbufp,
            tc.tile_pool(name="obuf", bufs=3) as obufp,
            tc.tile_pool(name="ps_a", bufs=2, space=bass.MemorySpace.PSUM) as ps_a,
            tc.tile_pool(name="ps_b", bufs=2, space=bass.MemorySpace.PSUM) as ps_b,
            tc.tile_pool(name="ps_m", bufs=1, space=bass.MemorySpace.PSUM) as ps_m,
            tc.tile_pool(name="ps_c", bufs=2, space=bass.MemorySpace.PSUM) as ps_c,
        ):
            # ---- constants ----
            m_aug = constp.tile([C + 1, C + 1], bf16)
            nc.sync.dma_start(m_aug[:], M_d[:])
            wvt = constp.tile([C + 1, C], bf16)
            nc.sync.dma_start(wvt[:], wv_d[:])
            w1t = constp.tile([C + 1, HID], bf16)
            nc.sync.dma_start(w1t[:], w1_d[:])
            w2ta = constp.tile([128, C], bf16)
            nc.sync.dma_start(w2ta[:], w2_d[0:128, :])
            w2tb = constp.tile([128, C], bf16)
            nc.sync.dma_start(w2tb[:], w2_d[128:256, :])
            b2c = constp.tile([C, 1], f32)
            nc.sync.dma_start(b2c[:], b2_d[:])
            ones128 = constp.tile([128, C], bf16)
            nc.gpsimd.memset(ones128[:], 1.0)

            # persistent y (post-attention residual), raster layout, flat
            y = ybufp.tile([C + 1, H * W], bf16)   # [65, 65536]
            nc.gpsimd.memset(y[C:C + 1, 0:H * W // 2], 1.0)
            nc.gpsimd.memset(y[C:C + 1, H * W // 2:], 1.0)

            # =================== phase 1: attention ===================
            for s in range(N_STRIPES):
                xin = xinp.tile([C, STRIPE_H, W], f32)
                nc.sync.dma_start(xin[:], x_d[:, s * GS:(s + 1) * GS, :])
                # window-major bf16 x with ones row: col w*64 + r*8 + c
                xw = xwinp.tile([C + 1, SPX], bf16)
                xw_v4 = xw[0:C, :].rearrange("p (w r c) -> p w r c", w=32, r=8, c=8)
                xin_v4 = xin[:].rearrange("p r (w c) -> p w r c", w=32, c=8)
                nc.vector.tensor_copy(xw_v4[:, 0:16, :, :], xin_v4[:, 0:16, :, :])
                nc.scalar.copy(xw_v4[:, 16:32, :, :], xin_v4[:, 16:32, :, :])
                if s < 2:
                    # xw pool has bufs=2; slots rotate so the ones row written
                    # in the first two iterations persists in both slots
                    nc.gpsimd.memset(xw[C:C + 1, :], 1.0)

                ybase = s * SPX
                for g in range(N_GROUPS):
                    c0 = g * GRP
                    xg = xw[:, c0:c0 + GRP]

                    # t2[j,k] = sum_i M_aug[i,j] x_aug[i,k]
                    t2p = ps_a.tile([C + 1, GRP], f32, tag="a")
                    nc.tensor.matmul(t2p[:], m_aug[:], xg, start=True, stop=True)
                    t2s = smallp.tile([C + 1, GRP], bf16, tag="t2s")
                    nc.scalar.copy(t2s[:], t2p[:])

                    # window pair p -> (w=p on partitions 0:64, w=p+4 on 64:128)
                    sp = ps_b.tile([128, GRP // 2], f32, tag="b")
                    vtp = ps_b.tile([128, GRP // 2], f32, tag="b")
                    for p in range(4):
                        for d in range(2):
                            w = p + 4 * d
                            xww = xw[:, c0 + C * w:c0 + C * (w + 1)]
                            tp = (0, 64) if d else (0, 0)
                            nc.tensor.matmul(sp[64 * d:64 * d + 64, C * p:C * (p + 1)],
                                             xww, t2s[:, C * w:C * (w + 1)],
                                             start=True, stop=True, tile_position=tp)
                            nc.tensor.matmul(vtp[64 * d:64 * d + 64, C * p:C * (p + 1)],
                                             xww, wvt[:], start=True, stop=True,
                                             tile_position=tp)

                    # P = exp(S/8); logits ~ N(0,1): no max subtraction needed
                    pexp = smallp.tile([128, GRP // 2], bf16, tag="pexp")
                    nc.scalar.activation(pexp[:], sp[:], EXP, scale=0.125)

                    vts = smallp.tile([128, GRP // 2], bf16, tag="vts")
                    nc.scalar.copy(vts[:], vtp[:])

                    # column sums: deck B re-decked to partitions 0:64 via
                    # tile_position=(64,0) (contraction rows 64-127, output cols 0-63)
                    sba = ps_m.tile([C, GRP // 2], f32, tag="sba")
                    sbb = ps_m.tile([C, GRP // 2], f32, tag="sbb")
                    nc.tensor.matmul(sba[:], ones128[0:64, :], pexp[0:64, :],
                                     start=True, stop=True, tile_position=(0, 0))
                    nc.tensor.matmul(sbb[:], ones128[64:128, :], pexp[64:128, :],
                                     start=True, stop=True, tile_position=(64, 0))
                    s_s = smallp.tile([C, GRP], f32, tag="ss")
                    nc.scalar.copy(s_s[:, 0:256], sba[:])
                    nc.vector.tensor_copy(s_s[:, 256:512], sbb[:])
                    rbc = smallp.tile([C, GRP], f32, tag="rbc")
                    nc.vector.reciprocal_approx_fast(rbc[:], s_s[:])

                    # out2[c,k] = sum_l vT[l,c] P[l,k]; deck B re-decked likewise
                    o2a = ps_c.tile([C, GRP // 2], f32, tag="c")
                    o2b = ps_c.tile([C, GRP // 2], f32, tag="c")
                    for p in range(4):
                        nc.tensor.matmul(o2a[:, C * p:C * (p + 1)],
                                         vts[0:64, C * p:C * (p + 1)],
                                         pexp[0:64, C * p:C * (p + 1)],
                                         start=True, stop=True, tile_position=(0, 0))
                        nc.tensor.matmul(o2b[:, C * p:C * (p + 1)],
                                         vts[64:128, C * p:C * (p + 1)],
                                         pexp[64:128, C * p:C * (p + 1)],
                                         start=True, stop=True, tile_position=(64, 0))

                    # normalize each deck, writing attn in group-raster order
                    attn = smallp.tile([C, GRP], bf16, tag="attn")
                    attn_v = attn[:].rearrange("p (r w c) -> p w r c", r=8, w=8, c=8)
                    nc.vector.tensor_mul(attn_v[:, 0:4, :, :], o2a[:],
                                         rbc[:, 0:256])
                    nc.vector.tensor_mul(attn_v[:, 4:8, :, :], o2b[:],
                                         rbc[:, 256:512])

                    # y = x + attn (bf16, step-1 innermost)
                    y_stripe = y[0:C, ybase:ybase + SPX].rearrange(
                        "p (r q) -> p r q", r=8, q=W)
                    nc.vector.tensor_add(
                        y_stripe[:, :, g * C:(g + 1) * C]
                        .rearrange("p r (w c) -> p r w c", w=8, c=8),
                        attn[:].rearrange("p (r w c) -> p r w c", r=8, w=8, c=8),
                        xw[0:C, c0:c0 + GRP].rearrange(
                            "p (w r c) -> p r w c", w=8, r=8, c=8))

            # =================== phase 2: FFN ===================
            for s in range(N_STRIPES):
                for hp in range(2):          # chunk pairs
                    bases = [s * SPX + (2 * hp + j) * GRP for j in range(2)]
                    ycs = [y[:, b:b + GRP] for b in bases]
                    h1p = [ps_a.tile([128, GRP], f32, tag="a", name=f"h1pa{hp}"),
                           ps_m.tile([128, GRP], f32, tag="sba", name=f"h1pb{hp}")]
                    h2p = [ps_b.tile([128, GRP], f32, tag="b", name=f"h2pa{hp}"),
                           ps_m.tile([128, GRP], f32, tag="sbb", name=f"h2pb{hp}")]
                    h1s = [hbufp.tile([128, GRP], bf16, tag="h1s", name=f"h1s{hp}_{j}") for j in range(2)]
                    h2s = [hbufp.tile([128, GRP], bf16, tag="h2s", name=f"h2s{hp}_{j}") for j in range(2)]
                    for j in range(2):
                        nc.tensor.matmul(h1p[j][:], w1t[:, 0:128], ycs[j],
                                         start=True, stop=True)
                    for j in range(2):
                        nc.tensor.matmul(h2p[j][:], w1t[:, 128:256], ycs[j],
                                         start=True, stop=True)
                    for j in range(2):
                        nc.scalar.activation(h1s[j][:], h1p[j][:], GELU)
                        nc.scalar.activation(h2s[j][:], h2p[j][:], GELU)
                    y2p = [ps_c.tile([C, GRP], f32, tag="c", name=f"y2p{hp}_{j}") for j in range(2)]
                    for j in range(2):
                        nc.tensor.matmul(y2p[j][:], w2ta[:], h1s[j][:],
                                         start=True, stop=False)
                    for j in range(2):
                        nc.tensor.matmul(y2p[j][:], w2tb[:], h2s[j][:],
                                         start=False, stop=True)
                    for j in range(2):
                        oc = obufp.tile([C, GRP], f32)
                        nc.vector.scalar_tensor_tensor(
                            oc[:], y2p[j][:], b2c[:], y[0:C, bases[j]:bases[j] + GRP],
                            op0=mybir.AluOpType.add, op1=mybir.AluOpType.add)
                        r0 = s * GS + (2 * hp + j) * 2
                        nc.sync.dma_start(out_d[:, r0:r0 + 2, :],
                                          oc[:].rearrange("p (r c) -> p r c", r=2, c=W))

    nc.compile()
    return nc
def _prep_weights(wq, bq, wk, bk, wv, bv, w1, b1, w2, b2):
    bf = ml_dtypes.bfloat16
    wq_aug = np.concatenate([wq.astype(np.float64),
                             bq.astype(np.float64)[:, None]], axis=1)  # [64,65]
    wk_aug = np.concatenate([wk.astype(np.float64),
                             bk.astype(np.float64)[:, None]], axis=1)
    m_aug = (wq_aug.T @ wk_aug).astype(np.float32).astype(bf)          # [65,65]
    wvt_aug = np.concatenate([wv.astype(np.float32).T,
                              bv.astype(np.float32)[None, :]], axis=0).astype(bf)
    w1t_aug = np.concatenate([w1.astype(np.float32).T,
                              b1.astype(np.float32)[None, :]], axis=0).astype(bf)
    w2t = np.ascontiguousarray(w2.astype(np.float32).T).astype(bf)     # [256,64]
    b2col = np.ascontiguousarray(b2.astype(np.float32)[:, None])       # [64,1]
    return m_aug, wvt_aug, w1t_aug, w2t, b2col


def kernel(x, wq, bq, wk, bk, wv, bv, w1, b1, w2, b2, _trace=False):
    from concourse.bass_utils import run_bass_kernel_spmd

    if "nc" not in _CACHE:
        _CACHE["nc"] = _build()
    nc = _CACHE["nc"]

    m_aug, wvt_aug, w1t_aug, w2t, b2col = _prep_weights(
        wq, bq, wk, bk, wv, bv, w1, b1, w2, b2)

    x = np.asarray(x, dtype=np.float32)
    B = x.shape[0]
    in_maps = []
    for i in range(8):
        in_maps.append({
            "x": np.ascontiguousarray(x[i % B]),
            "m_aug": m_aug, "wvt_aug": wvt_aug, "w1t_aug": w1t_aug,
            "w2t": w2t, "b2col": b2col,
        })

    res = run_bass_kernel_spmd(nc, in_maps, core_ids=list(range(8)),
                               trace=_trace)
    out = np.stack([np.asarray(res.results[i]["out"], dtype=np.float32)
                    for i in range(B)], axis=0)
    if _trace:
        return out, res
    return out
